# revision 40
# baseline (speedup 1.0000x reference)
"""Trainium2 Bass kernel for nn_MultiHeadedAttention_71425306132929.

Fused QKV projection + RoPE + causal/padding-masked SDPA + output projection.

Sharding: 8 cores = 2 batches x 4 head-groups (4 heads each).  Each core
computes, for its (batch, head-group):
    qkT = (Wq|Wk) @ query[b].T      (transposed layout: head-dim on partitions)
    RoPE on qT/kT via in-quadrant partition shuffle (head dims permuted
    host-side so RoPE partners are 16 partitions apart)
    scoresT[k,q] = kT.T-dot-qT per head (2 heads packed via PE row tiling)
    PT = exp(scoresT * 1/8)  (no max-subtraction needed: logits are O(1))
    causal masking: narrowed matmul/exp ranges + affine_select on the
    single in-diagonal 128x128 block
    padding mask: folded into v (zeroed rows) + an extra all-mask column that
    makes the attention-denominator fall out of the same matmul
    ohT = (v|m).T @ PT accumulated over key blocks -> unnormalized out + denom
    normalize: DVE reciprocal on the PSUM den row -> gpsimd partition
    broadcast -> fused multiply (no DRAM round-trips)
    yT_partial = WoutT.T @ ohT  (row-parallel out-projection, bf16 partials)
Host sums the 4 partial yT per batch.

The PE instruction stream is software-pipelined: the attention kb-loop
(ACT-exp paced) interleaves "feeder" units (next-chunk projections and
previous-chunk out-projection matmuls) so the PE never drains.
"""

import os
import sys

import numpy as np

sys.path.insert(0, "/opt/trn_rl_repo")

import concourse.bass as bass  # noqa: E402
import concourse.bacc as bacc  # noqa: E402
import concourse.tile as tile  # noqa: E402
from concourse import mybir  # noqa: E402

import ml_dtypes  # noqa: E402

BF16 = mybir.dt.bfloat16
F32 = mybir.dt.float32

B, S, DM, TD, H, HD = 2, 2048, 1024, 1024, 16, 64
NCORES = 8
NH = 4          # heads per core
NKB = S // 128  # 16 key blocks
NQC = S // 512  # 4 query chunks
KC = DM // 128  # 8 contraction chunks

# RoPE partner permutation: place original dim d so that partner(p) = p ^ 16
# (within a 32-partition quadrant, reachable by DVE stream_shuffle).
ROPE_PERM = []
for _p in range(64):
    q32, r32 = _p // 32, _p % 32
    ROPE_PERM.append(q32 * 16 + r32 if r32 < 16 else 32 + q32 * 16 + (r32 - 16))
ROPE_SGN = np.array([-1.0 if (p % 32) < 16 else 1.0 for p in range(64)], np.float32)
SHUF_MASK = [i ^ 16 for i in range(32)]

_CACHED = {}


def build_program():
    nc = bacc.Bacc(None, target_bir_lowering=False)
    qT_d = nc.declare_dram_parameter("qT", [DM, S], BF16, isOutput=False)
    wqk_d = nc.declare_dram_parameter("wqkT", [DM, 512], BF16, isOutput=False)
    wv_d = nc.declare_dram_parameter("wvT", [DM, 256], BF16, isOutput=False)
    cos_d = nc.declare_dram_parameter("cosT", [128, S], BF16, isOutput=False)
    sin_d = nc.declare_dram_parameter("sinT", [128, S], BF16, isOutput=False)
    mkv_d = nc.declare_dram_parameter("maskv", [128, NKB], F32, isOutput=False)
    wo_d = nc.declare_dram_parameter("woutT", [256, DM], BF16, isOutput=False)
    yT_d = nc.declare_dram_parameter("yT", [DM, S], BF16, isOutput=True)
    dscr = nc.dram_tensor("den_scratch", [16, 512], F32)
    dscr2 = nc.dram_tensor("rcp_scratch", [16, 512], F32)

    with tile.TileContext(nc) as tc:
        with (
            tc.tile_pool(name="const", bufs=1) as cpool,
            tc.tile_pool(name="work", bufs=1) as wpool,
            tc.tile_pool(name="rope", bufs=2) as rpool,
            tc.tile_pool(name="pt", bufs=6) as ptpool,
            tc.tile_pool(name="nrm", bufs=2) as npool,
            tc.tile_pool(name="yout", bufs=6) as ypool,
            tc.tile_pool(name="psA", bufs=2, space="PSUM") as psA,
            tc.tile_pool(name="psP", bufs=2, space="PSUM") as psP,
            tc.tile_pool(name="psO", bufs=2, space="PSUM") as psO,
        ):
            qT_sb = cpool.tile([128, KC, S], BF16, tag="qT")
            wqk_sb = cpool.tile([128, KC, 512], BF16, tag="wqk")
            wv_sb = cpool.tile([128, KC, 256], BF16, tag="wv")
            cos_sb = cpool.tile([128, S], BF16, tag="cos")
            sin_sb = cpool.tile([128, S], BF16, tag="sin")
            mkv_sb = cpool.tile([128, NKB], F32, tag="mkv")
            wo_sb = cpool.tile([128, 2, DM], BF16, tag="wo")

            qk_sb = wpool.tile([128, 4, S], BF16, tag="qk")
            vaug_sb = wpool.tile([128, NKB, 4, 128], BF16, tag="vaug")
            ohT_sb = wpool.tile([128, 2, S], BF16, tag="ohT")
            ones_sb = wpool.tile([128, 128], F32, tag="ones")

            wqk_r = wqk_d.rearrange("(c p) s -> p c s", p=128)
            qT_r = qT_d.rearrange("(c p) s -> p c s", p=128)
            yT_r = yT_d.rearrange("(c p) s -> p c s", p=128)

            # --- input DMA, in priority order (first-needed first) ---
            nc.sync.dma_start(mkv_sb[:], mkv_d[:])
            q0 = slice(0, 512)
            for kc in range(KC):
                nc.sync.dma_start(wqk_sb[:, kc, 256:384], wqk_r[:, kc, 256:384])
            for kc in range(KC):
                nc.sync.dma_start(qT_sb[:, kc, q0], qT_r[:, kc, q0])
            for kc in range(KC):
                nc.sync.dma_start(wqk_sb[:, kc, 0:128], wqk_r[:, kc, 0:128])
            nc.sync.dma_start(cos_sb[:, q0], cos_d[:, q0])
            nc.sync.dma_start(sin_sb[:, q0], sin_d[:, q0])
            nc.sync.dma_start(wv_sb[:], wv_d.rearrange("(c p) s -> p c s", p=128))
            for kc in range(KC):
                nc.sync.dma_start(wqk_sb[:, kc, 384:512], wqk_r[:, kc, 384:512])
                nc.sync.dma_start(wqk_sb[:, kc, 128:256], wqk_r[:, kc, 128:256])
            for qn in range(1, NQC):
                qs = slice(qn * 512, qn * 512 + 512)
                for kc in range(KC):
                    nc.sync.dma_start(qT_sb[:, kc, qs], qT_r[:, kc, qs])
                nc.sync.dma_start(cos_sb[:, qs], cos_d[:, qs])
                nc.sync.dma_start(sin_sb[:, qs], sin_d[:, qs])
            nc.sync.dma_start(wo_sb[:], wo_d.rearrange("(c p) s -> p c s", p=128))

            # --- vaug setup: mask columns + zero the junk strip read by the
            # narrowed odd-head attnv lhsT (cols 33:64 of odd slots) ---
            mkv_col = mkv_sb.rearrange("p (k o) -> p k o", o=1)
            nc.gpsimd.tensor_copy(vaug_sb[:, :, 0, 64:65], mkv_col)
            nc.gpsimd.tensor_copy(vaug_sb[:, :, 2, 64:65], mkv_col)
            nc.gpsimd.tensor_copy(vaug_sb[:, :, 1, 32:33], mkv_col)
            nc.gpsimd.tensor_copy(vaug_sb[:, :, 3, 32:33], mkv_col)
            nc.gpsimd.memset(vaug_sb[:, :, 0, 65:128], 0.0)
            nc.gpsimd.memset(vaug_sb[:, :, 2, 65:128], 0.0)
            nc.gpsimd.memset(vaug_sb[:, :, 1, 0:32], 0.0)
            nc.gpsimd.memset(vaug_sb[:, :, 3, 0:32], 0.0)
            nc.gpsimd.memset(vaug_sb[:, :, 1, 33:64], 0.0)
            nc.gpsimd.memset(vaug_sb[:, :, 3, 33:64], 0.0)
            nc.vector.memset(ones_sb[:], 1.0)

            # ---------- feeder unit machinery ----------
            # Emission order = per-engine execution order; attention kb-loops
            # pop one unit per kb so projection matmuls fill the PE bubbles of
            # the exp-paced stream.
            hi = []  # prep units (qk/v) -- deadline-driven
            lo = []  # outproj units -- pure filler, drained late
            lo_floor = [0]  # units held back to fill the final norm's latency
            deferred = []  # y DMAs of the reserved tail-fill units

            def pop_one():
                if hi:
                    hi.pop(0)["fn"]()
                elif len(lo) > lo_floor[0]:
                    lo.pop(0)["fn"]()

            def fill_lo(n):
                for _ in range(n):
                    if lo:
                        lo.pop(0)["fn"]()

            def flush(pred=None):
                i = 0
                while i < len(hi):
                    if pred is None or pred(hi[i]):
                        hi.pop(i)["fn"]()
                    else:
                        i += 1
                if pred is None:
                    while lo:
                        lo.pop(0)["fn"]()

            # ---------- building blocks ----------
            def emit_qk_units(mt, qn):
                """project + rope one [128, 512] chunk of q or k (2 heads);
                4 units of 2 matmuls each; RoPE rides on the last unit."""
                qsl = slice(qn * 512, qn * 512 + 512)
                state = {}

                def mms(k0, k1, start, stop):
                    def fn():
                        if start:
                            state["ps"] = psP.tile([128, 512], F32, tag="psP", name="psP")
                        ps = state["ps"]
                        for kc in range(k0, k1):
                            nc.tensor.matmul(
                                ps[:],
                                lhsT=wqk_sb[:, kc, mt * 128:(mt + 1) * 128],
                                rhs=qT_sb[:, kc, qsl],
                                start=(kc == k0 and start),
                                stop=(kc == k1 - 1 and stop),
                                skip_group_check=True,
                            )
                        if stop:
                            ps = state["ps"]
                            shuf = rpool.tile([128, 512], F32, tag="shuf", name="shuf")
                            nc.vector.stream_shuffle(shuf[:], ps[:], mask=SHUF_MASK)
                            t1 = rpool.tile([128, 512], BF16, tag="t1", name="t1")
                            nc.vector.tensor_mul(t1[:], ps[:], cos_sb[:, qsl])
                            t2 = rpool.tile([128, 512], BF16, tag="t2", name="t2")
                            nc.vector.tensor_mul(t2[:], shuf[:], sin_sb[:, qsl])
                            nc.vector.tensor_add(qk_sb[:, mt, qsl], t1[:], t2[:])
                    return fn

                return [
                    {"kind": "qk", "key": (mt, qn), "fn": mms(0, 2, True, False)},
                    {"kind": "qk", "key": (mt, qn), "fn": mms(2, 4, False, False)},
                    {"kind": "qk", "key": (mt, qn), "fn": mms(4, 6, False, False)},
                    {"kind": "qk", "key": (mt, qn), "fn": mms(6, 8, False, True)},
                ]

            def emit_v_units(st):
                """project + mask one [128 keys, 4*64] v block into v_aug;
                2 units of 4 matmuls."""
                state = {}

                def mms(k0, k1, start, stop):
                    def fn():
                        if start:
                            state["ps"] = psP.tile([128, 512], F32, tag="psP", name="psP")
                        psv = state["ps"][:, 0:256]
                        for kc in range(k0, k1):
                            nc.tensor.matmul(
                                psv,
                                lhsT=qT_sb[:, kc, st * 128:(st + 1) * 128],
                                rhs=wv_sb[:, kc, :],
                                start=(kc == k0 and start),
                                stop=(kc == k1 - 1 and stop),
                                skip_group_check=True,
                            )
                        if stop:
                            psv_h = psv.rearrange("p (h d) -> p h d", h=4)
                            msk = mkv_sb[:, st:st + 1]
                            nc.vector.tensor_scalar_mul(
                                vaug_sb[:, st, 0:4:2, 0:64], psv_h[:, 0:4:2, :], msk)
                            nc.vector.tensor_scalar_mul(
                                vaug_sb[:, st, 1:4:2, 64:128], psv_h[:, 1:4:2, :], msk)
                    return fn

                return [
                    {"kind": "v", "key": st, "fn": mms(0, 4, True, False)},
                    {"kind": "v", "key": st, "fn": mms(4, 8, False, True)},
                ]

            def outproj_units(qc, dual_engine=False, defer_dma=None):
                """row-parallel out-projection of one 512-query chunk.  With
                dual_engine the PSUM->SBUF copies alternate Scalar/Vector --
                used at the tail where ACT is otherwise idle.  With defer_dma
                (a list), the y write-out DMAs are appended to it instead of
                emitted -- used for units that fill the final normalize
                latency, so the big writes don't delay its small DMAs."""
                qsl = slice(qc * 512, qc * 512 + 512)
                state = {}
                units = []
                for mt in range(8):
                    def fn(mt=mt):
                        ps = psP.tile([128, 512], F32, tag="psP", name="psPy")
                        for kc2 in range(2):
                            nc.tensor.matmul(
                                ps[:],
                                lhsT=wo_sb[:, kc2, mt * 128:(mt + 1) * 128],
                                rhs=ohT_sb[:, kc2, qsl],
                                start=(kc2 == 0),
                                stop=(kc2 == 1),
                                skip_group_check=True,
                            )
                        if mt % 2 == 0:
                            state["y"] = ypool.tile([128, 2, 512], BF16, tag="y", name="y")
                        if dual_engine and mt % 2 == 0:
                            nc.scalar.copy(state["y"][:, 0, :], ps[:])
                        else:
                            nc.vector.tensor_copy(state["y"][:, mt % 2, :], ps[:])
                        if mt % 2 == 1:
                            y = state["y"]
                            def dma(y=y, mt=mt):
                                nc.sync.dma_start(yT_r[:, mt - 1:mt + 1, qsl], y[:])
                            if defer_dma is not None:
                                defer_dma.append(dma)
                            else:
                                dma()
                    units.append({"kind": "op", "key": (qc, mt), "fn": fn})
                return units

            def attn(pair, qc, fast_norm=False, tail_fill=False):
                """attention for one (head-pair, query-chunk): kb-pipelined
                scores -> exp -> (diag mask) -> attnv, one feeder pop per kb."""
                nkb = 4 * qc + 4
                qmt, kmt = pair, 2 + pair
                # flush any still-queued producers of this chunk's q/k
                flush(lambda u: u["kind"] == "qk" and u["key"][0] in (qmt, kmt)
                      and u["key"][1] == qc)
                if pair == 0 and qc >= 1:
                    # the flushed q/k units' RoPE tails must drain before this
                    # chunk's first scores; fill that window with independent
                    # units (the other pair's projections) instead of stalling
                    for _ in range(4):
                        pop_one()
                oT = [psO.tile([128, 512], F32, tag="psO", name=f"oT{pair}{qc}{_h}")
                      for _h in range(2)]
                pts = [None] * nkb
                cos_ = [0] * nkb

                def attnv(kb):
                    co = cos_[kb]
                    pt = pts[kb]
                    for h in range(2):
                        slot = 2 * pair + h
                        nc.tensor.matmul(
                            oT[h][:, co:512],
                            lhsT=vaug_sb[:, kb, slot, :],
                            rhs=pt[:, h * 512 + co:(h + 1) * 512],
                            start=(kb == 0),
                            stop=(kb == nkb - 1),
                            skip_group_check=True,
                        )

                def scores(kb):
                    joff = kb - 4 * qc
                    co = max(joff, 0) * 128
                    cos_[kb] = co
                    ksl = slice(kb * 128, kb * 128 + 128)
                    # v_aug for this kb must be emitted already
                    flush(lambda u: u["kind"] == "v" and u["key"] <= kb)
                    st_ps = psA.tile([128, 1024], F32, tag="psA", name="stps")
                    for h in range(2):
                        pr = slice(64 * h, 64 * h + 64)
                        nc.tensor.matmul(
                            st_ps[:, h * 512 + co:(h + 1) * 512],
                            lhsT=qk_sb[pr, kmt, ksl],
                            rhs=qk_sb[pr, qmt, qc * 512 + co:qc * 512 + 512],
                            start=True,
                            stop=True,
                            skip_group_check=True,
                        )
                    pt = ptpool.tile([128, 1024], BF16, tag="pt", name="pt")
                    pts[kb] = pt
                    if co == 0:
                        nc.scalar.activation(
                            pt[:], st_ps[:],
                            mybir.ActivationFunctionType.Exp, scale=0.125)
                    else:
                        for h in range(2):
                            nc.scalar.activation(
                                pt[:, h * 512 + co:(h + 1) * 512],
                                st_ps[:, h * 512 + co:(h + 1) * 512],
                                mybir.ActivationFunctionType.Exp, scale=0.125)
                    if joff >= 0:
                        for h in range(2):
                            nc.gpsimd.affine_select(
                                pt[:, h * 512 + co:h * 512 + co + 128],
                                pt[:, h * 512 + co:h * 512 + co + 128],
                                pattern=[[1, 128]],
                                compare_op=mybir.AluOpType.is_ge,
                                fill=0.0,
                                base=0,
                                channel_multiplier=-1,
                            )

                # kbs processed in batches of two: scores for both land before
                # the pair of attnv's, so the PE waits (and drains its
                # pipeline) once per batch instead of once per kb
                for kb in range(0, nkb, 2):
                    scores(kb)
                    scores(kb + 1)
                    if kb < 2:
                        # extra filler early in the round: gives the previous
                        # pair's normalize chain slack before attnv(0) blocks
                        # the in-order PE stream on the PSUM pool
                        pop_one()
                    pop_one()
                    if kb >= 2:
                        attnv(kb - 2)
                        attnv(kb - 1)
                    pop_one()
                attnv(nkb - 2)
                attnv(nkb - 1)

                # normalize: copy live PSUM rows out (frees the banks fast),
                # then 1/den via either a partition-packed reciprocal (DMA
                # reshape through DRAM -- SBUF APs cannot remap
                # partition<->free; deep latency but fully hidden mid-kernel)
                # or, for the very last round, an on-chip serial reciprocal +
                # PE-matmul broadcast (higher engine cost, ~4x lower latency).
                qsl = slice(qc * 512, qc * 512 + 512)
                osb0 = npool.tile([128, 512], F32, tag="osb0", name="osb0")
                osb1 = npool.tile([128, 512], F32, tag="osb1", name="osb1")
                if fast_norm:
                    # all-on-chip low-latency variant for the final round: PE
                    # broadcasts both den rows into one PSUM tile, a single
                    # parallel-lane approx reciprocal inverts both, and the
                    # muls read the attnv PSUM directly (no DMA anywhere).
                    if pair == 1:
                        # tail: ACT is idle, keep DVE free for the muls
                        nc.scalar.copy(osb0[64:65, :], oT[0][64:65, :])
                        nc.scalar.copy(osb1[32:33, :], oT[1][32:33, :])
                    else:
                        # mid-round: ACT is exp-saturated, DVE has headroom
                        nc.vector.tensor_copy(osb0[64:65, :], oT[0][64:65, :])
                        nc.vector.tensor_copy(osb1[32:33, :], oT[1][32:33, :])
                    bc = psP.tile([128, 512], F32, tag="psP", name="bcps")
                    nc.tensor.matmul(
                        bc[0:64, :], lhsT=ones_sb[64:65, 0:64], rhs=osb0[64:65, :],
                        start=True, stop=True, skip_group_check=True)
                    nc.tensor.matmul(
                        bc[64:128, :], lhsT=ones_sb[32:33, 0:64], rhs=osb1[32:33, :],
                        start=True, stop=True, skip_group_check=True)
                    rr = npool.tile([128, 512], F32, tag="rr", name="rr")
                    nc.vector.reciprocal_approx_fast(rr[:], bc[:])
                    nc.vector.tensor_mul(ohT_sb[0:64, pair, qsl], oT[0][0:64, :], rr[0:64, :])
                    nc.vector.tensor_mul(ohT_sb[64:128, pair, qsl], oT[1][64:128, :], rr[64:128, :])
                    return
                nc.vector.tensor_copy(osb0[64:65, :], oT[0][64:65, :])
                nc.vector.tensor_copy(osb1[32:33, :], oT[1][32:33, :])
                nc.vector.tensor_copy(osb0[0:64, :], oT[0][0:64, :])
                nc.vector.tensor_copy(osb1[64:128, :], oT[1][64:128, :])
                if False:
                    pass
                else:
                    base = (pair * NQC + qc) * 2
                    nc.sync.dma_start(dscr[base:base + 1, :], osb0[64:65, :])
                    nc.sync.dma_start(dscr[base + 1:base + 2, :], osb1[32:33, :])
                    rcp = npool.tile([128, 8], F32, tag="rcp", name="rcp")
                    nc.sync.dma_start(
                        rcp[:], dscr[base:base + 2, :].rearrange("a (p f) -> (a p) f", f=8))
                    if tail_fill:
                        fill_lo(6)
                    rcp2 = npool.tile([128, 8], F32, tag="rcp2", name="rcp2")
                    nc.vector.reciprocal(rcp2[:], rcp[:])
                    nc.sync.dma_start(
                        dscr2[base:base + 2, :].rearrange("a (p f) -> (a p) f", f=8), rcp2[:])
                    bc = npool.tile([128, 512], F32, tag="bc", name="bc")
                    nc.sync.dma_start(
                        bc[0:64, :], dscr2[base:base + 1, :].to_broadcast((64, 512)))
                    nc.sync.dma_start(
                        bc[64:128, :], dscr2[base + 1:base + 2, :].to_broadcast((64, 512)))
                    if tail_fill:
                        fill_lo(16)
                nc.vector.tensor_mul(ohT_sb[0:64, pair, qsl], osb0[0:64, :], bc[0:64, :])
                nc.vector.tensor_mul(ohT_sb[64:128, pair, qsl], osb1[64:128, :], bc[64:128, :])
                if tail_fill:
                    for dma in deferred:
                        dma()
                    deferred.clear()

            # ---------- main schedule ----------
            def push_prep(qn):
                # qk units first: popping them early in the round lets their
                # RoPE tails drain long before the next round's scores need
                # qk_sb (flushing them at the boundary exposes that latency)
                for mt in (2, 0, 3, 1):
                    hi.extend(emit_qk_units(mt, qn))
                for st in range(4 * qn, 4 * qn + 4):
                    hi.extend(emit_v_units(st))

            # pair-0 projections of the first chunk run directly; pair-1's
            # ride the feeder queue so attention starts as early as possible
            for u in emit_qk_units(2, 0) + emit_qk_units(0, 0):
                u["fn"]()
            for u in emit_v_units(0) + emit_v_units(1):
                u["fn"]()
            for mt in (3, 1):
                hi.extend(emit_qk_units(mt, 0))
            hi.extend(emit_v_units(2))
            hi.extend(emit_v_units(3))

            for qc in range(NQC):
                if qc + 1 < NQC:
                    push_prep(qc + 1)
                attn(0, qc, fast_norm=(qc == NQC - 1))
                attn(1, qc, fast_norm=(qc == NQC - 1))
                lo.extend(outproj_units(qc, dual_engine=(qc == NQC - 1)))
                # queued q/k units for the next chunk's pair-0 must land
                # before its attn starts; v units flush inside attn.
            flush()

    nc.compile()
    return nc


def make_in_maps(query, W_in, W_out, sin_q, cos_q, attn_mask):
    bf = ml_dtypes.bfloat16
    cosT = np.asarray(cos_q, np.float32)[0, 0].T  # [64, S]
    sinT = np.asarray(sin_q, np.float32)[0, 0].T
    cosT_p = cosT[ROPE_PERM]
    sinT_p = sinT[ROPE_PERM] * ROPE_SGN[:, None]
    cos2 = np.concatenate([cosT_p, cosT_p], 0).astype(bf)    # [128, S]
    sin2 = np.concatenate([sinT_p, sinT_p], 0).astype(bf)
    W_in = np.asarray(W_in, np.float32)
    W_out = np.asarray(W_out, np.float32)
    query = np.asarray(query, np.float32)
    attn_mask = np.asarray(attn_mask)

    in_maps = []
    for c in range(NCORES):
        b, g = c // 4, c % 4
        heads = range(4 * g, 4 * g + 4)
        qrows = np.concatenate([W_in[h * 64:(h + 1) * 64][ROPE_PERM] for h in heads])
        krows = np.concatenate([W_in[TD + h * 64:TD + (h + 1) * 64][ROPE_PERM] for h in heads])
        vrows = np.concatenate([W_in[2 * TD + h * 64:2 * TD + (h + 1) * 64] for h in heads])
        tcols = np.concatenate([np.arange(h * 64, (h + 1) * 64) for h in heads])
        in_maps.append({
            "qT": np.ascontiguousarray(query[b].T).astype(bf),
            "wqkT": np.ascontiguousarray(np.concatenate([qrows, krows], 0).T).astype(bf),
            "wvT": np.ascontiguousarray(vrows.T).astype(bf),
            "cosT": cos2,
            "sinT": sin2,
            "maskv": np.ascontiguousarray(
                attn_mask[b].astype(np.float32).reshape(NKB, 128).T),
            "woutT": np.ascontiguousarray(W_out[:, tcols].T).astype(bf),
        })
    return in_maps


def _ensure_ntff_hook():
    """The image's antenv lacks axon_hooks; supply it so trace=True works."""
    try:
        from antenv.axon_hooks import get_axon_ntff_profile_hook  # noqa: F401
        return
    except ImportError:
        pass
    import types

    if "/root/.axon_site" not in sys.path:
        sys.path.insert(0, "/root/.axon_site")
    from trn_agent_boot.trn_boot import _ntff_profile_via_ctypes

    hook = _ntff_profile_via_ctypes("/opt/axon/libaxon_pjrt.so")
    mod = types.ModuleType("antenv.axon_hooks")
    mod._hook = hook
    mod.get_axon_ntff_profile_hook = lambda: mod._hook
    mod.set_axon_ntff_profile_hook = lambda h: setattr(mod, "_hook", h)
    sys.modules["antenv.axon_hooks"] = mod
    import antenv

    antenv.axon_hooks = mod


def kernel(query, W_in, W_out, sin_q, cos_q, attn_mask):
    if "nc" not in _CACHED:
        _CACHED["nc"] = build_program()
    nc = _CACHED["nc"]
    in_maps = make_in_maps(query, W_in, W_out, sin_q, cos_q, attn_mask)

    from concourse.bass_utils import run_bass_kernel_spmd

    trace = bool(os.environ.get("KERNEL_PROFILE"))
    if trace:
        try:
            _ensure_ntff_hook()
        except Exception as e:  # profiling is best-effort
            print(f"ntff hook unavailable: {e}")
            trace = False
    try:
        res = run_bass_kernel_spmd(nc, in_maps, list(range(NCORES)), trace=trace)
    except Exception:
        if not trace:
            raise
        res = run_bass_kernel_spmd(nc, in_maps, list(range(NCORES)), trace=False)
    _CACHED["last_result"] = res

    y = np.zeros((B, S, DM), np.float32)
    for c in range(NCORES):
        y[c // 4] += np.asarray(res.results[c]["yT"], np.float32).T
    return y
